# revision 18
# baseline (speedup 1.0000x reference)
"""BPMLL loss kernel for Trainium2, data-parallel over 8 NeuronCores.

Reference computation (per sample row i of c [B, L], y [B, L] in {0,1}):
    pos_i  = sum_l y_il * exp(-c_il)
    neg_i  = sum_l (1 - y_il) * exp(c_il)
    Sy_i   = sum_l y_il
    loss_i = pos_i * neg_i / (Sy_i * (L - Sy_i))
    out    = mean_i loss_i                      (scalar, float32)

Device strategy: shard the batch dim across 8 cores (2048 rows each). The
label masking is folded into the exponent: with s = M*y - c and M = 128,
    exp(-s)     = exp(c - M*y)     -> (1-y)*exp(c)   (y=1 underflows to 0)
    exp(s - M)  = exp(-c + M*(y-1))-> y*exp(-c)      (y=0 underflows to 0)
so ScalarE's fused activation-with-accumulate computes each masked row sum
in a single pass. Per [128, 1024] tile the device does: one DVE
scalar_tensor_tensor (s = y*M - c), two ScalarE exp+accum passes, and one
DVE reduce_sum over y. Each core emits [3, 128, 16] row statistics
(pos, neg, Sy); the host finishes the per-row division and global mean.
"""

import numpy as np

B, L = 16384, 1024
N_CORES = 8
BS = B // N_CORES  # 2048 rows per core
P = 128
NSEG = BS // P  # 16 tiles of [128, L] per core
MASK = 128.0


def _build_nc():
    import concourse.bacc as bacc
    import concourse.mybir as mybir
    from concourse.tile import TileContext

    f32 = mybir.dt.float32
    i32 = mybir.dt.int32

    nc = bacc.Bacc()
    c_in = nc.dram_tensor("c", [BS, L], f32, kind="ExternalInput")
    y_in = nc.dram_tensor("y", [BS, L], i32, kind="ExternalInput")
    stats = nc.dram_tensor("stats", [3, P, NSEG], f32, kind="ExternalOutput")

    c_tiled = c_in[:].rearrange("(n p) l -> n p l", p=P)
    y_tiled = y_in[:].rearrange("(n p) l -> n p l", p=P)

    with TileContext(nc) as tc:
        with (
            tc.tile_pool(name="io", bufs=5) as io,
            tc.tile_pool(name="work", bufs=3) as work,
            tc.tile_pool(name="accs", bufs=1) as accs,
        ):
            pos = accs.tile([P, NSEG], f32)
            neg = accs.tile([P, NSEG], f32)
            ysum = accs.tile([P, NSEG], f32)
            neg_mask = accs.tile([P, 1], f32)
            nc.vector.memset(neg_mask[:], -MASK)

            for i in range(NSEG):
                ctile = io.tile([P, L], f32, tag="c")
                ytile = io.tile([P, L], i32, tag="y")
                nc.gpsimd.dma_start(ctile[:], c_tiled[i])
                nc.gpsimd.dma_start(ytile[:], y_tiled[i])

                s = work.tile([P, L], f32, tag="s")
                nc.vector.scalar_tensor_tensor(
                    s[:],
                    ytile[:],
                    MASK,
                    ctile[:],
                    mybir.AluOpType.mult,
                    mybir.AluOpType.subtract,
                )
                nc.vector.reduce_sum(
                    ysum[:, i : i + 1], ytile[:], axis=mybir.AxisListType.X
                )
                scr = work.tile([P, L], f32, tag="scr")
                nc.scalar.activation(
                    scr[:],
                    s[:],
                    mybir.ActivationFunctionType.Exp,
                    scale=-1.0,
                    accum_out=neg[:, i : i + 1],
                )
                scr2 = work.tile([P, L], f32, tag="scr2")
                nc.scalar.activation(
                    scr2[:],
                    s[:],
                    mybir.ActivationFunctionType.Exp,
                    bias=neg_mask[:],
                    scale=1.0,
                    accum_out=pos[:, i : i + 1],
                )


            nc.sync.dma_start(stats[0], pos[:])
            nc.sync.dma_start(stats[1], neg[:])
            nc.sync.dma_start(stats[2], ysum[:])

    nc.finalize()
    return nc


def _run(nc, in_maps, **kwargs):
    from concourse.bass_utils import run_bass_kernel_spmd

    return run_bass_kernel_spmd(nc, in_maps, list(range(N_CORES)), **kwargs)


def kernel(c, y, _bench_kwargs=None, _bench_result=None):
    c = np.ascontiguousarray(np.asarray(c, dtype=np.float32))
    y = np.ascontiguousarray(np.asarray(y, dtype=np.int32))
    assert c.shape == (B, L) and y.shape == (B, L)

    nc = _build_nc()
    in_maps = [
        {"c": c[k * BS : (k + 1) * BS], "y": y[k * BS : (k + 1) * BS]}
        for k in range(N_CORES)
    ]
    res = _run(nc, in_maps, **(_bench_kwargs or {}))
    if _bench_result is not None:
        _bench_result.append(res)

    stats = np.stack([r["stats"] for r in res.results])  # [8, 3, 128, 16]
    pos = stats[:, 0].astype(np.float64)
    neg = stats[:, 1].astype(np.float64)
    sy = stats[:, 2].astype(np.float64)
    loss = pos * neg / (sy * (L - sy))
    return np.asarray(loss.mean(), dtype=np.float32)


# revision 19
# speedup vs baseline: 1.1738x; 1.1738x over previous
"""BPMLL loss kernel for Trainium2, data-parallel over 8 NeuronCores.

Reference computation (per sample row i of c [B, L], y [B, L] in {0,1}):
    pos_i  = sum_l y_il * exp(-c_il)
    neg_i  = sum_l (1 - y_il) * exp(c_il)
    Sy_i   = sum_l y_il
    loss_i = pos_i * neg_i / (Sy_i * (L - Sy_i))
    out    = mean_i loss_i                      (scalar, float32)

Device strategy: shard the batch dim across 8 cores (2048 rows each). The
label masking is folded into the exponent: with s = M*y - c and M = 128,
    exp(-s)     = exp(c - M*y)     -> (1-y)*exp(c)   (y=1 underflows to 0)
    exp(s - M)  = exp(-c + M*(y-1))-> y*exp(-c)      (y=0 underflows to 0)
so ScalarE's fused activation-with-accumulate computes each masked row sum
in a single pass.

The host interleaves c (f32) and y (int32, bit-viewed as f32) into one
tensor so each [128, 1024] row-tile pair arrives in a single 1 MB DMA;
the kernel bitcasts the y half back to int32 on-chip. Per tile the device
does: one DVE scalar_tensor_tensor (s = y*M - c), one DVE reduce_sum over
y, and two ScalarE exp+accum passes. Each core emits [3, 128, 16] row
statistics (pos, neg, Sy); the host finishes the tiny per-row division
and the global mean in float64.
"""

import numpy as np

B, L = 16384, 1024
N_CORES = 8
BS = B // N_CORES  # 2048 rows per core
P = 128
NSEG = BS // P  # 16 tiles of [128, L] per core
MASK = 128.0
DGE = "gpsimd"  # which engine issues the input loads: "gpsimd" or "sync"
IO_BUFS = 4


def _build_nc():
    import concourse.bacc as bacc
    import concourse.mybir as mybir
    from concourse.tile import TileContext

    f32 = mybir.dt.float32
    i32 = mybir.dt.int32

    nc = bacc.Bacc()
    cy_in = nc.dram_tensor("cy", [NSEG, 2, P, L], f32, kind="ExternalInput")
    stats = nc.dram_tensor("stats", [3, P, NSEG], f32, kind="ExternalOutput")

    with TileContext(nc) as tc:
        with (
            tc.tile_pool(name="io", bufs=IO_BUFS) as io,
            tc.tile_pool(name="work", bufs=3) as work,
            tc.tile_pool(name="accs", bufs=1) as accs,
        ):
            pos = accs.tile([P, NSEG], f32)
            neg = accs.tile([P, NSEG], f32)
            ysum = accs.tile([P, NSEG], f32)
            neg_mask = accs.tile([P, 1], f32)
            nc.vector.memset(neg_mask[:], -MASK)

            dma_eng = nc.gpsimd if DGE == "gpsimd" else nc.sync
            for i in range(NSEG):
                t = io.tile([P, 2, L], f32, tag="cy")
                dma_eng.dma_start(t[:], cy_in[i].rearrange("t p l -> p t l"))
                c_ap = t[:, 0, :]
                y_ap = t[:, 1, :].bitcast(i32)

                s = work.tile([P, L], f32, tag="s")
                nc.vector.scalar_tensor_tensor(
                    s[:],
                    y_ap,
                    MASK,
                    c_ap,
                    mybir.AluOpType.mult,
                    mybir.AluOpType.subtract,
                )
                nc.vector.reduce_sum(
                    ysum[:, i : i + 1], y_ap, axis=mybir.AxisListType.X
                )
                scr = work.tile([P, L], f32, tag="scr")
                nc.scalar.activation(
                    scr[:],
                    s[:],
                    mybir.ActivationFunctionType.Exp,
                    scale=-1.0,
                    accum_out=neg[:, i : i + 1],
                )
                scr2 = work.tile([P, L], f32, tag="scr2")
                nc.scalar.activation(
                    scr2[:],
                    s[:],
                    mybir.ActivationFunctionType.Exp,
                    bias=neg_mask[:],
                    scale=1.0,
                    accum_out=pos[:, i : i + 1],
                )

            nc.sync.dma_start(stats[0], pos[:])
            nc.sync.dma_start(stats[1], neg[:])
            nc.sync.dma_start(stats[2], ysum[:])

    nc.finalize()
    return nc


def _run(nc, in_maps, **kwargs):
    from concourse.bass_utils import run_bass_kernel_spmd

    return run_bass_kernel_spmd(nc, in_maps, list(range(N_CORES)), **kwargs)


def kernel(c, y, _bench_kwargs=None, _bench_result=None):
    c = np.asarray(c, dtype=np.float32)
    y = np.asarray(y, dtype=np.int32)
    assert c.shape == (B, L) and y.shape == (B, L)

    # Interleave per [128, L] row-tile: cy[k, i, 0] = c rows, cy[k, i, 1] =
    # y rows bit-viewed as f32, so each tile pair is one contiguous 1 MB DMA.
    cyv = np.empty((N_CORES, NSEG, 2, P, L), np.float32)
    cyv[:, :, 0] = np.ascontiguousarray(c).reshape(N_CORES, NSEG, P, L)
    cyv[:, :, 1] = np.ascontiguousarray(y).view(np.float32).reshape(
        N_CORES, NSEG, P, L
    )

    nc = _build_nc()
    in_maps = [{"cy": cyv[k]} for k in range(N_CORES)]
    res = _run(nc, in_maps, **(_bench_kwargs or {}))
    if _bench_result is not None:
        _bench_result.append(res)

    stats = np.stack([r["stats"] for r in res.results])  # [8, 3, 128, 16]
    pos = stats[:, 0].astype(np.float64)
    neg = stats[:, 1].astype(np.float64)
    sy = stats[:, 2].astype(np.float64)
    loss = pos * neg / (sy * (L - sy))
    return np.asarray(loss.mean(), dtype=np.float32)
